# revision 24
# baseline (speedup 1.0000x reference)
"""Trainium2 Bass kernel for a 2-layer GRU decoder (B=128, T=512, H=512).

Sharding: data-parallel on batch across 8 cores (16 batch elems/core).

Architecture (v5): both GRU layers run in lockstep on every core, with
layer 6 lagging layer 5 by one 32-step chunk.  All per-step gate math
is FUSED across the two layers into single double-width ops (a layer
dimension in the PSUM / xw / h tiles), halving DVE op count and
semaphore traffic; the z|r matmul groups of both layers run before the
ih groups so the z|r gate math overlaps the ih matmul stream.  All
intermediates (xW5, g5, xW6, g6) live in SBUF rings -- no DRAM
roundtrips.  xW bulk matmuls + the dense output head are computed
per-chunk inside the same software-pipelined loop.  The phase-1
contraction is reduced 320->80 via  mask*concat(z,x2) @ W5
   = (mask*x2) @ W5[x2-rows] + (mask*onehot) @ (z@W5[z-rows]).
h is carried in bf16 (the z-gate convex blend damps rounding noise).
"""

import numpy as np
import ml_dtypes

B, T, LAT, F2, H = 128, 512, 256, 64, 512
G3 = 3 * H              # 1536
NCORES = 8
BL = B // NCORES        # 16 batch / core
NT = T * BL             # 8192 flat (t,b) cols per core
KH = H // 128           # 4 hidden-dim chunks
NCH = G3 // 128         # 12 gate-dim chunks (0-3 z, 4-7 r, 8-11 ih)
C = 32                  # timesteps per chunk
NCHK = T // C           # 16 chunks

bf16 = ml_dtypes.bfloat16
_CACHE = {}


def _build(bd_val, has_brh):
    import concourse.bass as bass
    import concourse.tile as tile
    import concourse.mybir as mybir
    from concourse import bacc
    from concourse.bass import ds

    f32 = mybir.dt.float32
    bf = mybir.dt.bfloat16
    AF = mybir.ActivationFunctionType
    OP = mybir.AluOpType
    ET = mybir.EngineType

    nc = bacc.Bacc(None, target_bir_lowering=False, debug=False)

    xt_d = nc.dram_tensor("xt_d", [128, NT], bf, kind="ExternalInput")
    w5a_d = nc.dram_tensor("w5a_d", [128, G3], bf, kind="ExternalInput")
    u5_d = nc.dram_tensor("u5_d", [KH, 128, G3], bf, kind="ExternalInput")
    w6_d = nc.dram_tensor("w6_d", [KH, 128, G3], bf, kind="ExternalInput")
    u6_d = nc.dram_tensor("u6_d", [KH, 128, G3], bf, kind="ExternalInput")
    b5_d = nc.dram_tensor("b5_d", [128, NCH], f32, kind="ExternalInput")
    b6_d = nc.dram_tensor("b6_d", [128, NCH], f32, kind="ExternalInput")
    br5_d = nc.dram_tensor("br5_d", [128, KH], f32, kind="ExternalInput")
    br6_d = nc.dram_tensor("br6_d", [128, KH], f32, kind="ExternalInput")
    wd_d = nc.dram_tensor("wd_d", [128, KH], bf, kind="ExternalInput")
    dm_d = nc.dram_tensor("dm_d", [1, NT], f32, kind="ExternalInput")
    out_d = nc.dram_tensor("out_d", [1, NT], f32, kind="ExternalOutput")

    with tile.TileContext(nc) as tc:
        import contextlib
        stack = contextlib.ExitStack()
        with stack:
            pp = stack.enter_context(tc.tile_pool(name="persist", bufs=1))
            u5_sb = pp.tile([128, KH, G3], bf, tag="u5")
            u6_sb = pp.tile([128, KH, G3], bf, tag="u6")
            w6_sb = pp.tile([128, KH, G3], bf, tag="w6")
            w5a_sb = pp.tile([128, G3], bf, tag="w5a")
            b5_sb = pp.tile([128, NCH], f32, tag="b5")
            b6_sb = pp.tile([128, NCH], f32, tag="b6")
            wd_sb = pp.tile([128, KH], bf, tag="wd")
            # fused rings (ring index = L5-chunk parity; layer dim: 0=L5
            # chunk c, 1=L6 chunk c-1)
            xa_sb = [pp.tile([128, 512], bf, tag=f"xa{r}", name=f"xa{r}")
                     for r in range(2)]
            xwb_sb = [pp.tile([128, C, 2, NCH, BL], bf, tag=f"xwb{r}",
                              name=f"xwb{r}") for r in range(2)]
            hb_sb = [pp.tile([128, 2, KH, C, BL], bf, tag=f"hb{r}",
                             name=f"hb{r}") for r in range(2)]
            dmc_sb = [pp.tile([1, 512], f32, tag=f"dm{r}", name=f"dmc{r}")
                      for r in range(2)]
            dec_sb = [pp.tile([1, 512], f32, tag=f"dec{r}", name=f"dec{r}")
                      for r in range(2)]

            nc.sync.dma_start(u5_sb[:], u5_d.ap().rearrange("k p g -> p k g"))
            nc.sync.dma_start(u6_sb[:], u6_d.ap().rearrange("k p g -> p k g"))
            nc.sync.dma_start(w6_sb[:], w6_d.ap().rearrange("k p g -> p k g"))
            nc.sync.dma_start(w5a_sb[:], w5a_d.ap()[:])
            nc.sync.dma_start(b5_sb[:], b5_d.ap()[:])
            nc.sync.dma_start(b6_sb[:], b6_d.ap()[:])
            nc.sync.dma_start(wd_sb[:], wd_d.ap()[:])
            nc.vector.memset(hb_sb[0][:], 0.0)
            nc.vector.memset(hb_sb[1][:], 0.0)
            if has_brh:
                brhb_sb = pp.tile([128, 2, KH], f32, tag="brhb")
                nc.sync.dma_start(brhb_sb[:, 0, :], br5_d.ap()[:])
                nc.sync.dma_start(brhb_sb[:, 1, :], br6_d.ap()[:])

            psb = stack.enter_context(
                tc.tile_pool(name="psbulk", bufs=2, space="PSUM"))
            psd = stack.enter_context(
                tc.tile_pool(name="psdense", bufs=1, space="PSUM"))
            prec = stack.enter_context(
                tc.tile_pool(name="prec", bufs=3, space="PSUM"))
            wk = stack.enter_context(tc.tile_pool(name="wk", bufs=3))

            def bulk5(ring, colbase):
                """xW5 chunk c: xaug @ W5aug + b5 -> xwb[ring] layer 0."""
                nc.sync.dma_start(
                    xa_sb[ring][:], xt_d.ap()[:, ds(colbase, 512)])
                for cc in range(NCH):
                    ps = psb.tile([128, 512], f32, tag="psb")
                    nc.tensor.matmul(
                        ps[:], w5a_sb[:, cc * 128:(cc + 1) * 128],
                        xa_sb[ring][:], start=True, stop=True,
                    )
                    nc.scalar.activation(
                        xwb_sb[ring][:, :, 0, cc, :],
                        ps[:].rearrange("p (t b) -> p t b", b=BL),
                        AF.Identity, bias=b5_sb[:, cc:cc + 1],
                    )

            def bulk6(rh):
                """xW6 for L6 chunk c': g5 (hb[rh] layer 0) @ W6 + b6
                -> xwb[1-rh] layer 1 (consumed with L5 chunk c'+1)."""
                for cc in range(NCH):
                    ps = psb.tile([128, 512], f32, tag="psb")
                    for k in range(KH):
                        nc.tensor.matmul(
                            ps[:], w6_sb[:, k, cc * 128:(cc + 1) * 128],
                            hb_sb[rh][:, 0, k, :, :]
                            .rearrange("p t b -> p (t b)"),
                            start=(k == 0), stop=(k == KH - 1),
                        )
                    nc.scalar.activation(
                        xwb_sb[1 - rh][:, :, 1, cc, :],
                        ps[:].rearrange("p (t b) -> p t b", b=BL),
                        AF.Identity, bias=b6_sb[:, cc:cc + 1],
                    )

            def dense(rh, colbase):
                """dec chunk: tanh(g6 @ Wd + bd) * dec_mask -> out_d.
                g6 chunk = hb[rh] layer 1."""
                nc.sync.dma_start(
                    dmc_sb[rh][:], dm_d.ap()[:, ds(colbase, 512)])
                ps = psd.tile([1, 512], f32, tag="psd")
                for k in range(KH):
                    nc.tensor.matmul(
                        ps[:], wd_sb[:, k:k + 1],
                        hb_sb[rh][:, 1, k, :, :]
                        .rearrange("p t b -> p (t b)"),
                        start=(k == 0), stop=(k == KH - 1),
                    )
                nc.scalar.activation(
                    dec_sb[rh][:], ps[:], AF.Tanh, bias=float(bd_val))
                nc.gpsimd.tensor_tensor(
                    dec_sb[rh][:], dec_sb[rh][:], dmc_sb[rh][:],
                    op=OP.mult)
                nc.sync.dma_start(
                    out_d.ap()[:, ds(colbase, 512)], dec_sb[rh][:])

            def rec_chunk(r, ls):
                """One 32-step chunk; ls = layer slots: [0] solo L5,
                [1] solo L6, [0, 1] fused pair (L5 chunk c + L6 c-1)."""
                l0, n = ls[0], len(ls)
                for uu in range(C):
                    ps = prec.tile([128, 2, 3, KH, BL], f32, tag="psrec")

                    def hp(li, k):
                        if uu > 0:
                            return hb_sb[r][:, li, k, uu - 1, :]
                        return hb_sb[1 - r][:, li, k, C - 1, :]

                    u_of = {0: u5_sb, 1: u6_sb}
                    # z|r groups of all layers first, then ih groups:
                    # the z|r gate math overlaps the ih matmul stream.
                    for g in range(2):
                        for li in ls:
                            for cg in range(KH):
                                for k in range(KH):
                                    nc.tensor.matmul(
                                        ps[:, li, g, cg, :],
                                        u_of[li][:, k,
                                                 g * 512 + cg * 128:
                                                 g * 512 + (cg + 1) * 128],
                                        hp(li, k),
                                        start=(k == 0),
                                        stop=(k == KH - 1),
                                        skip_group_check=True,
                                    )
                    for li in ls:
                        for cg in range(KH):
                            for k in range(KH):
                                nc.tensor.matmul(
                                    ps[:, li, 2, cg, :],
                                    u_of[li][:, k,
                                             1024 + cg * 128:
                                             1024 + (cg + 1) * 128],
                                    hp(li, k),
                                    start=(k == 0),
                                    stop=(k == KH - 1),
                                    skip_group_check=True,
                                )

                    xw = xwb_sb[r][:, uu, l0:l0 + n, :, :]
                    if uu > 0:
                        hprev = hb_sb[r][:, l0:l0 + n, :, uu - 1, :]
                    else:
                        hprev = hb_sb[1 - r][:, l0:l0 + n, :, C - 1, :]

                    szr = wk.tile([128, 2, 2 * KH, BL], bf, tag="szr")
                    nc.vector.tensor_tensor(
                        szr[:, 0:n],
                        xw[:, :, 0:8, :],
                        ps[:, l0:l0 + n, 0:2, :, :]
                        .rearrange("p l g k b -> p l (g k) b"),
                        op=OP.add,
                    )
                    gzr = wk.tile([128, 2, 2 * KH, BL], bf, tag="gzr")
                    nc.vector.tensor_scalar(
                        gzr[:, 0:n], szr[:, 0:n], 0.0, 1.0,
                        op0=OP.max, op1=OP.min,
                    )
                    q = wk.tile([128, 2, KH, BL], bf, tag="q")
                    if has_brh:
                        nc.vector.scalar_tensor_tensor(
                            q[:, 0:n], ps[:, l0:l0 + n, 2, :, :], 1.0,
                            brhb_sb[:, l0:l0 + n, :]
                            .rearrange("p l k -> p l k 1").broadcast(3, BL),
                            op0=OP.mult, op1=OP.add,
                        )
                        nc.vector.tensor_tensor(
                            q[:, 0:n], gzr[:, 0:n, KH:2 * KH, :], q[:, 0:n],
                            op=OP.mult)
                    else:
                        nc.vector.tensor_tensor(
                            q[:, 0:n], gzr[:, 0:n, KH:2 * KH, :],
                            ps[:, l0:l0 + n, 2, :, :], op=OP.mult)
                    hpre = wk.tile([128, 2, KH, BL], bf, tag="hpre")
                    nc.vector.tensor_tensor(
                        hpre[:, 0:n], q[:, 0:n], xw[:, :, 8:12, :],
                        op=OP.add)
                    hh = wk.tile([128, 2, KH, BL], bf, tag="hh")
                    nc.scalar.activation(hh[:, 0:n], hpre[:, 0:n], AF.Tanh)
                    # blend prep on gpsimd, in the tanh/q shadow
                    vg = wk.tile([128, 2, KH, BL], bf, tag="vg")
                    nc.gpsimd.tensor_scalar(
                        vg[:, 0:n], gzr[:, 0:n, 0:KH, :], -1.0, 1.0,
                        op0=OP.mult, op1=OP.add,
                    )
                    ug = wk.tile([128, 2, KH, BL], bf, tag="ug")
                    nc.gpsimd.tensor_tensor(
                        ug[:, 0:n], gzr[:, 0:n, 0:KH, :], hprev,
                        op=OP.mult)
                    ee = wk.tile([128, 2, KH, BL], bf, tag="ee")
                    nc.vector.tensor_tensor(
                        ee[:, 0:n], vg[:, 0:n], hh[:, 0:n], op=OP.mult)
                    nc.vector.tensor_tensor(
                        hb_sb[r][:, l0:l0 + n, :, uu, :],
                        ug[:, 0:n], ee[:, 0:n], op=OP.add)

            # ---------------- prologue ----------------
            bulk5(0, 0)
            bulk5(1, 512)
            rec_chunk(0, [0])                    # L5 chunk 0
            bulk6(0)                             # xw6 c0 -> xwb[1] slot 1
            bulk5(0, 1024)                       # chunk 2
            rec_chunk(1, [0, 1])                 # L5 c1 + L6 c0
            bulk6(1)                             # xw6 c1 -> xwb[0] slot 1
            bulk5(1, 3 * 512)                    # chunk 3
            dense(1, 0)                          # dec chunk 0

            # -------- steady loop: L5 chunks 2..13, L6 1..12 ----------
            with tc.For_i(
                2 * 512, 14 * 512, 2 * 512,
                hint_engines=(ET.PE, ET.DVE, ET.Activation, ET.Pool),
            ) as iv:
                rec_chunk(0, [0, 1])             # L5 c (even) + L6 c-1
                bulk6(0)
                bulk5(0, iv + 2 * 512)
                dense(0, iv - 512)
                rec_chunk(1, [0, 1])             # L5 c+1 + L6 c
                bulk6(1)
                bulk5(1, iv + 3 * 512)
                dense(1, iv)

            # ---------------- epilogue ----------------
            rec_chunk(0, [0, 1])                 # L5 c14 + L6 c13
            bulk6(0)
            dense(0, 13 * 512)
            rec_chunk(1, [0, 1])                 # L5 c15 + L6 c14
            bulk6(1)
            dense(1, 14 * 512)
            rec_chunk(0, [1])                    # L6 c15
            dense(0, 15 * 512)

    nc.compile()
    return nc


def _prep(inputs):
    """Host-side: shard on batch, permute/pad/cast into device layouts."""
    z = np.asarray(inputs["z"], np.float32)
    x2 = np.asarray(inputs["train_input_two"], np.float32)
    masks = np.asarray(inputs["masks"], np.float32)
    dmasks = np.asarray(inputs["dec_masks"], np.float32)
    W5 = np.asarray(inputs["W5"], np.float32)
    U5 = np.asarray(inputs["U5"], np.float32)
    bi5 = np.asarray(inputs["bi5"], np.float32)
    br5 = np.asarray(inputs["br5"], np.float32)
    W6 = np.asarray(inputs["W6"], np.float32)
    U6 = np.asarray(inputs["U6"], np.float32)
    bi6 = np.asarray(inputs["bi6"], np.float32)
    br6 = np.asarray(inputs["br6"], np.float32)
    Wd = np.asarray(inputs["Wd"], np.float32)
    bd = np.asarray(inputs["bd"], np.float32)

    def scale_w(W):  # scale z,r columns by 0.2 (hard-sigmoid prescale)
        Ws = W.copy()
        Ws[:, : 2 * H] *= 0.2
        return Ws

    def pack_w(W, kdim):  # [D,G3] -> [kdim,128,G3] bf16 (zero-padded)
        D = W.shape[0]
        Wp = np.zeros((kdim * 128, G3), np.float32)
        Wp[:D] = W
        return np.ascontiguousarray(Wp.reshape(kdim, 128, G3).astype(bf16))

    def pack_bias(bi, br):  # xw-path bias, [128, NCH] (partition, chunk)
        bt = np.empty(G3, np.float32)
        bt[: 2 * H] = 0.2 * (bi[: 2 * H] + br[: 2 * H]) + 0.5
        bt[2 * H:] = bi[2 * H:]
        return np.ascontiguousarray(bt.reshape(NCH, 128).T)

    W5s = scale_w(W5)
    u5p = pack_w(scale_w(U5), KH)
    w6p = pack_w(scale_w(W6), KH)
    u6p = pack_w(scale_w(U6), KH)
    b5p = pack_bias(bi5, br5)
    b6p = pack_bias(bi6, br6)
    brh5 = np.ascontiguousarray(br5[2 * H:].reshape(KH, 128).T)
    brh6 = np.ascontiguousarray(br6[2 * H:].reshape(KH, 128).T)
    has_brh = bool(np.any(brh5) or np.any(brh6))
    wdp = np.ascontiguousarray(Wd[:, 0].reshape(KH, 128).T.astype(bf16))

    in_maps = []
    for cidx in range(NCORES):
        sl = slice(cidx * BL, (cidx + 1) * BL)
        # augmented input: rows 0:64 = (mask*x2)^T, 64:80 = mask*onehot(b)
        xm = x2[sl] * masks[sl]                       # [BL,T,64]
        XT = np.zeros((128, T, BL), np.float32)
        XT[:F2] = xm.transpose(2, 1, 0)
        mk = masks[sl, :, 0]                          # [BL,T]
        for b in range(BL):
            XT[F2 + b, :, b] = mk[b]
        # augmented W5: rows 0:64 = W5s[x2 rows], 64:80 = z @ W5s[z rows]
        W5a = np.zeros((128, G3), np.float32)
        W5a[:F2] = W5s[LAT:]
        W5a[F2:F2 + BL] = z[sl] @ W5s[:LAT]
        dmc = dmasks[sl, :, 0].T.reshape(NT)          # flat t*BL+b
        in_maps.append({
            "xt_d": np.ascontiguousarray(
                XT.reshape(128, NT).astype(bf16)),
            "w5a_d": np.ascontiguousarray(W5a.astype(bf16)),
            "u5_d": u5p, "w6_d": w6p, "u6_d": u6p,
            "b5_d": b5p, "b6_d": b6p,
            "br5_d": brh5, "br6_d": brh6,
            "wd_d": wdp,
            "dm_d": np.ascontiguousarray(dmc.reshape(1, NT)),
        })
    return in_maps, has_brh, float(bd.reshape(-1)[0])


def kernel(**inputs):
    from concourse.bass_utils import run_bass_kernel_spmd

    in_maps, has_brh, bd_val = _prep(inputs)
    key = (has_brh, bd_val)
    if key not in _CACHE:
        _CACHE[key] = _build(bd_val, has_brh)
    nc = _CACHE[key]
    res = run_bass_kernel_spmd(nc, in_maps, core_ids=list(range(NCORES)))
    out = np.empty((B, T, 1), np.float32)
    for cidx in range(NCORES):
        flat = res.results[cidx]["out_d"].reshape(NT)  # flat = t*BL + b
        out[cidx * BL:(cidx + 1) * BL, :, 0] = flat.reshape(T, BL).T
    return out


# revision 28
# speedup vs baseline: 1.1382x; 1.1382x over previous
"""Trainium2 Bass kernel for a 2-layer GRU decoder (B=128, T=512, H=512).

Sharding: data-parallel on batch across 8 cores (16 batch elems/core).

Architecture (v5): both GRU layers run in lockstep on every core, with
layer 6 lagging layer 5 by one 32-step chunk.  All per-step gate math
is FUSED across the two layers into single double-width ops (a layer
dimension in the PSUM / xw / h tiles), halving DVE op count and
semaphore traffic; the z|r matmul groups of both layers run before the
ih groups so the z|r gate math overlaps the ih matmul stream.  All
intermediates (xW5, g5, xW6, g6) live in SBUF rings -- no DRAM
roundtrips.  xW bulk matmuls + the dense output head are computed
per-chunk inside the same software-pipelined loop.  The phase-1
contraction is reduced 320->80 via  mask*concat(z,x2) @ W5
   = (mask*x2) @ W5[x2-rows] + (mask*onehot) @ (z@W5[z-rows]).
h is carried in bf16 (the z-gate convex blend damps rounding noise).
"""

import numpy as np
import ml_dtypes

B, T, LAT, F2, H = 128, 512, 256, 64, 512
G3 = 3 * H              # 1536
NCORES = 8
BL = B // NCORES        # 16 batch / core
NT = T * BL             # 8192 flat (t,b) cols per core
KH = H // 128           # 4 hidden-dim chunks
NCH = G3 // 128         # 12 gate-dim chunks (0-3 z, 4-7 r, 8-11 ih)
C = 32                  # timesteps per chunk
NCHK = T // C           # 16 chunks

bf16 = ml_dtypes.bfloat16
_CACHE = {}


def _build(bd_val, has_brh):
    import concourse.bass as bass
    import concourse.tile as tile
    import concourse.mybir as mybir
    from concourse import bacc
    from concourse.bass import ds

    f32 = mybir.dt.float32
    bf = mybir.dt.bfloat16
    AF = mybir.ActivationFunctionType
    OP = mybir.AluOpType
    ET = mybir.EngineType

    nc = bacc.Bacc(None, target_bir_lowering=False, debug=False)

    xt_d = nc.dram_tensor("xt_d", [128, NT], bf, kind="ExternalInput")
    w5a_d = nc.dram_tensor("w5a_d", [128, G3], bf, kind="ExternalInput")
    u5_d = nc.dram_tensor("u5_d", [KH, 128, G3], bf, kind="ExternalInput")
    w6_d = nc.dram_tensor("w6_d", [KH, 128, G3], bf, kind="ExternalInput")
    u6_d = nc.dram_tensor("u6_d", [KH, 128, G3], bf, kind="ExternalInput")
    b5_d = nc.dram_tensor("b5_d", [128, NCH], f32, kind="ExternalInput")
    b6_d = nc.dram_tensor("b6_d", [128, NCH], f32, kind="ExternalInput")
    br5_d = nc.dram_tensor("br5_d", [128, KH], f32, kind="ExternalInput")
    br6_d = nc.dram_tensor("br6_d", [128, KH], f32, kind="ExternalInput")
    wd_d = nc.dram_tensor("wd_d", [128, KH], bf, kind="ExternalInput")
    dm_d = nc.dram_tensor("dm_d", [1, NT], f32, kind="ExternalInput")
    out_d = nc.dram_tensor("out_d", [1, NT], f32, kind="ExternalOutput")

    with tile.TileContext(nc) as tc:
        import contextlib
        stack = contextlib.ExitStack()
        with stack:
            pp = stack.enter_context(tc.tile_pool(name="persist", bufs=1))
            u5_sb = pp.tile([128, KH, G3], bf, tag="u5")
            u6_sb = pp.tile([128, KH, G3], bf, tag="u6")
            w6_sb = pp.tile([128, KH, G3], bf, tag="w6")
            w5a_sb = pp.tile([128, G3], bf, tag="w5a")
            b5_sb = pp.tile([128, NCH], f32, tag="b5")
            b6_sb = pp.tile([128, NCH], f32, tag="b6")
            wd_sb = pp.tile([128, KH], bf, tag="wd")
            # fused rings (ring index = L5-chunk parity; layer dim: 0=L5
            # chunk c, 1=L6 chunk c-1)
            xa_sb = [pp.tile([128, 512], bf, tag=f"xa{r}", name=f"xa{r}")
                     for r in range(2)]
            xwb_sb = [pp.tile([128, C, 2, NCH, BL], bf, tag=f"xwb{r}",
                              name=f"xwb{r}") for r in range(2)]
            hb_sb = [pp.tile([128, 2, KH, C, BL], bf, tag=f"hb{r}",
                             name=f"hb{r}") for r in range(2)]
            dmc_sb = [pp.tile([1, 512], f32, tag=f"dm{r}", name=f"dmc{r}")
                      for r in range(2)]
            dec_sb = [pp.tile([1, 512], f32, tag=f"dec{r}", name=f"dec{r}")
                      for r in range(2)]

            nc.sync.dma_start(u5_sb[:], u5_d.ap().rearrange("k p g -> p k g"))
            nc.sync.dma_start(u6_sb[:], u6_d.ap().rearrange("k p g -> p k g"))
            nc.sync.dma_start(w6_sb[:], w6_d.ap().rearrange("k p g -> p k g"))
            nc.sync.dma_start(w5a_sb[:], w5a_d.ap()[:])
            nc.sync.dma_start(b5_sb[:], b5_d.ap()[:])
            nc.sync.dma_start(b6_sb[:], b6_d.ap()[:])
            nc.sync.dma_start(wd_sb[:], wd_d.ap()[:])
            nc.vector.memset(hb_sb[0][:], 0.0)
            nc.vector.memset(hb_sb[1][:], 0.0)
            if has_brh:
                brhb_sb = pp.tile([128, 2, KH], f32, tag="brhb")
                nc.sync.dma_start(brhb_sb[:, 0, :], br5_d.ap()[:])
                nc.sync.dma_start(brhb_sb[:, 1, :], br6_d.ap()[:])

            psb = stack.enter_context(
                tc.tile_pool(name="psbulk", bufs=2, space="PSUM"))
            psd = stack.enter_context(
                tc.tile_pool(name="psdense", bufs=1, space="PSUM"))
            # one pool per layer; zr and ih are SEPARATE tiles so the
            # tile-granular dependency tracker lets each gate op fire as
            # soon as its own matmul group finishes (not the full stream)
            prec = [stack.enter_context(
                tc.tile_pool(name=f"prec{li}", bufs=1, space="PSUM"))
                for li in range(2)]
            wk = stack.enter_context(tc.tile_pool(name="wk", bufs=3))

            def bulk5(ring, colbase):
                """xW5 chunk c: xaug @ W5aug + b5 -> xwb[ring] layer 0."""
                nc.sync.dma_start(
                    xa_sb[ring][:], xt_d.ap()[:, ds(colbase, 512)])
                for cc in range(NCH):
                    ps = psb.tile([128, 512], f32, tag="psb")
                    nc.tensor.matmul(
                        ps[:], w5a_sb[:, cc * 128:(cc + 1) * 128],
                        xa_sb[ring][:], start=True, stop=True,
                    )
                    nc.scalar.activation(
                        xwb_sb[ring][:, :, 0, cc, :],
                        ps[:].rearrange("p (t b) -> p t b", b=BL),
                        AF.Identity, bias=b5_sb[:, cc:cc + 1],
                    )

            def bulk6(rh):
                """xW6 for L6 chunk c': g5 (hb[rh] layer 0) @ W6 + b6
                -> xwb[1-rh] layer 1 (consumed with L5 chunk c'+1)."""
                for cc in range(NCH):
                    ps = psb.tile([128, 512], f32, tag="psb")
                    for k in range(KH):
                        nc.tensor.matmul(
                            ps[:], w6_sb[:, k, cc * 128:(cc + 1) * 128],
                            hb_sb[rh][:, 0, k, :, :]
                            .rearrange("p t b -> p (t b)"),
                            start=(k == 0), stop=(k == KH - 1),
                        )
                    nc.scalar.activation(
                        xwb_sb[1 - rh][:, :, 1, cc, :],
                        ps[:].rearrange("p (t b) -> p t b", b=BL),
                        AF.Identity, bias=b6_sb[:, cc:cc + 1],
                    )

            def dense(rh, colbase):
                """dec chunk: tanh(g6 @ Wd + bd) * dec_mask -> out_d.
                g6 chunk = hb[rh] layer 1."""
                nc.sync.dma_start(
                    dmc_sb[rh][:], dm_d.ap()[:, ds(colbase, 512)])
                ps = psd.tile([1, 512], f32, tag="psd")
                for k in range(KH):
                    nc.tensor.matmul(
                        ps[:], wd_sb[:, k:k + 1],
                        hb_sb[rh][:, 1, k, :, :]
                        .rearrange("p t b -> p (t b)"),
                        start=(k == 0), stop=(k == KH - 1),
                    )
                nc.scalar.activation(
                    dec_sb[rh][:], ps[:], AF.Tanh, bias=float(bd_val))
                nc.gpsimd.tensor_tensor(
                    dec_sb[rh][:], dec_sb[rh][:], dmc_sb[rh][:],
                    op=OP.mult)
                nc.sync.dma_start(
                    out_d.ap()[:, ds(colbase, 512)], dec_sb[rh][:])

            def rec_chunk(r, ls):
                """One 32-step chunk; ls = layer slots: [0] solo L5,
                [1] solo L6, [0, 1] pair (L5 chunk c + L6 chunk c-1).

                Matmul stream order zr5, ih5, zr6, ih6; each group has
                its own PSUM tile so its gate math fires the moment the
                group finishes and overlaps the rest of the stream."""
                u_of = {0: u5_sb, 1: u6_sb}
                for uu in range(C):

                    def hp(li, k):
                        if uu > 0:
                            return hb_sb[r][:, li, k, uu - 1, :]
                        return hb_sb[1 - r][:, li, k, C - 1, :]

                    pzr, pih = {}, {}
                    for li in ls:
                        pzr[li] = prec[li].tile(
                            [128, 2, KH, BL], f32, tag="pszr",
                            name=f"pszr{li}")
                        pih[li] = prec[li].tile(
                            [128, KH, BL], f32, tag="psih",
                            name=f"psih{li}")
                        for g in range(2):
                            for cg in range(KH):
                                for k in range(KH):
                                    nc.tensor.matmul(
                                        pzr[li][:, g, cg, :],
                                        u_of[li][:, k,
                                                 g * 512 + cg * 128:
                                                 g * 512 + (cg + 1) * 128],
                                        hp(li, k),
                                        start=(k == 0),
                                        stop=(k == KH - 1),
                                        skip_group_check=True,
                                    )
                        for cg in range(KH):
                            for k in range(KH):
                                nc.tensor.matmul(
                                    pih[li][:, cg, :],
                                    u_of[li][:, k,
                                             1024 + cg * 128:
                                             1024 + (cg + 1) * 128],
                                    hp(li, k),
                                    start=(k == 0),
                                    stop=(k == KH - 1),
                                    skip_group_check=True,
                                )

                    for li in ls:
                        xw = xwb_sb[r][:, uu, li, :, :]
                        if uu > 0:
                            hprev = hb_sb[r][:, li, :, uu - 1, :]
                        else:
                            hprev = hb_sb[1 - r][:, li, :, C - 1, :]

                        szr = wk.tile([128, 2 * KH, BL], bf,
                                      tag=f"szr{li}", name=f"szr{li}")
                        nc.vector.tensor_tensor(
                            szr[:], xw[:, 0:8, :],
                            pzr[li][:].rearrange("p g k b -> p (g k) b"),
                            op=OP.add,
                        )
                        gzr = wk.tile([128, 2 * KH, BL], bf,
                                      tag=f"gzr{li}", name=f"gzr{li}")
                        nc.vector.tensor_scalar(
                            gzr[:], szr[:], 0.0, 1.0,
                            op0=OP.max, op1=OP.min,
                        )
                        q = wk.tile([128, KH, BL], bf,
                                    tag=f"q{li}", name=f"q{li}")
                        if has_brh:
                            nc.vector.scalar_tensor_tensor(
                                q[:], pih[li][:], 1.0,
                                brhb_sb[:, li, :]
                                .rearrange("p k -> p k 1").broadcast(2, BL),
                                op0=OP.mult, op1=OP.add,
                            )
                            nc.vector.tensor_tensor(
                                q[:], gzr[:, KH:2 * KH, :], q[:],
                                op=OP.mult)
                        else:
                            nc.vector.tensor_tensor(
                                q[:], gzr[:, KH:2 * KH, :], pih[li][:],
                                op=OP.mult)
                        hpre = wk.tile([128, KH, BL], bf,
                                       tag=f"hpre{li}", name=f"hpre{li}")
                        nc.vector.tensor_tensor(
                            hpre[:], q[:], xw[:, 8:12, :], op=OP.add)
                        hh = wk.tile([128, KH, BL], bf,
                                     tag=f"hh{li}", name=f"hh{li}")
                        nc.scalar.activation(hh[:], hpre[:], AF.Tanh)
                        # blend prep on gpsimd, in the tanh/q shadow
                        vg = wk.tile([128, KH, BL], bf,
                                     tag=f"vg{li}", name=f"vg{li}")
                        nc.gpsimd.tensor_scalar(
                            vg[:], gzr[:, 0:KH, :], -1.0, 1.0,
                            op0=OP.mult, op1=OP.add,
                        )
                        ug = wk.tile([128, KH, BL], bf,
                                     tag=f"ug{li}", name=f"ug{li}")
                        nc.gpsimd.tensor_tensor(
                            ug[:], gzr[:, 0:KH, :], hprev, op=OP.mult)
                        ee = wk.tile([128, KH, BL], bf,
                                     tag=f"ee{li}", name=f"ee{li}")
                        nc.vector.tensor_tensor(
                            ee[:], vg[:], hh[:], op=OP.mult)
                        nc.vector.tensor_tensor(
                            hb_sb[r][:, li, :, uu, :], ug[:], ee[:],
                            op=OP.add)

            # ---------------- prologue ----------------
            bulk5(0, 0)
            bulk5(1, 512)
            rec_chunk(0, [0])                    # L5 chunk 0
            bulk6(0)                             # xw6 c0 -> xwb[1] slot 1
            bulk5(0, 1024)                       # chunk 2
            rec_chunk(1, [0, 1])                 # L5 c1 + L6 c0
            bulk6(1)                             # xw6 c1 -> xwb[0] slot 1
            bulk5(1, 3 * 512)                    # chunk 3
            dense(1, 0)                          # dec chunk 0

            # -------- steady loop: L5 chunks 2..13, L6 1..12 ----------
            with tc.For_i(
                2 * 512, 14 * 512, 2 * 512,
                hint_engines=(ET.PE, ET.DVE, ET.Activation, ET.Pool),
            ) as iv:
                rec_chunk(0, [0, 1])             # L5 c (even) + L6 c-1
                bulk6(0)
                bulk5(0, iv + 2 * 512)
                dense(0, iv - 512)
                rec_chunk(1, [0, 1])             # L5 c+1 + L6 c
                bulk6(1)
                bulk5(1, iv + 3 * 512)
                dense(1, iv)

            # ---------------- epilogue ----------------
            rec_chunk(0, [0, 1])                 # L5 c14 + L6 c13
            bulk6(0)
            dense(0, 13 * 512)
            rec_chunk(1, [0, 1])                 # L5 c15 + L6 c14
            bulk6(1)
            dense(1, 14 * 512)
            rec_chunk(0, [1])                    # L6 c15
            dense(0, 15 * 512)

    nc.compile()
    return nc


def _prep(inputs):
    """Host-side: shard on batch, permute/pad/cast into device layouts."""
    z = np.asarray(inputs["z"], np.float32)
    x2 = np.asarray(inputs["train_input_two"], np.float32)
    masks = np.asarray(inputs["masks"], np.float32)
    dmasks = np.asarray(inputs["dec_masks"], np.float32)
    W5 = np.asarray(inputs["W5"], np.float32)
    U5 = np.asarray(inputs["U5"], np.float32)
    bi5 = np.asarray(inputs["bi5"], np.float32)
    br5 = np.asarray(inputs["br5"], np.float32)
    W6 = np.asarray(inputs["W6"], np.float32)
    U6 = np.asarray(inputs["U6"], np.float32)
    bi6 = np.asarray(inputs["bi6"], np.float32)
    br6 = np.asarray(inputs["br6"], np.float32)
    Wd = np.asarray(inputs["Wd"], np.float32)
    bd = np.asarray(inputs["bd"], np.float32)

    def scale_w(W):  # scale z,r columns by 0.2 (hard-sigmoid prescale)
        Ws = W.copy()
        Ws[:, : 2 * H] *= 0.2
        return Ws

    def pack_w(W, kdim):  # [D,G3] -> [kdim,128,G3] bf16 (zero-padded)
        D = W.shape[0]
        Wp = np.zeros((kdim * 128, G3), np.float32)
        Wp[:D] = W
        return np.ascontiguousarray(Wp.reshape(kdim, 128, G3).astype(bf16))

    def pack_bias(bi, br):  # xw-path bias, [128, NCH] (partition, chunk)
        bt = np.empty(G3, np.float32)
        bt[: 2 * H] = 0.2 * (bi[: 2 * H] + br[: 2 * H]) + 0.5
        bt[2 * H:] = bi[2 * H:]
        return np.ascontiguousarray(bt.reshape(NCH, 128).T)

    W5s = scale_w(W5)
    u5p = pack_w(scale_w(U5), KH)
    w6p = pack_w(scale_w(W6), KH)
    u6p = pack_w(scale_w(U6), KH)
    b5p = pack_bias(bi5, br5)
    b6p = pack_bias(bi6, br6)
    brh5 = np.ascontiguousarray(br5[2 * H:].reshape(KH, 128).T)
    brh6 = np.ascontiguousarray(br6[2 * H:].reshape(KH, 128).T)
    has_brh = bool(np.any(brh5) or np.any(brh6))
    wdp = np.ascontiguousarray(Wd[:, 0].reshape(KH, 128).T.astype(bf16))

    in_maps = []
    for cidx in range(NCORES):
        sl = slice(cidx * BL, (cidx + 1) * BL)
        # augmented input: rows 0:64 = (mask*x2)^T, 64:80 = mask*onehot(b)
        xm = x2[sl] * masks[sl]                       # [BL,T,64]
        XT = np.zeros((128, T, BL), np.float32)
        XT[:F2] = xm.transpose(2, 1, 0)
        mk = masks[sl, :, 0]                          # [BL,T]
        for b in range(BL):
            XT[F2 + b, :, b] = mk[b]
        # augmented W5: rows 0:64 = W5s[x2 rows], 64:80 = z @ W5s[z rows]
        W5a = np.zeros((128, G3), np.float32)
        W5a[:F2] = W5s[LAT:]
        W5a[F2:F2 + BL] = z[sl] @ W5s[:LAT]
        dmc = dmasks[sl, :, 0].T.reshape(NT)          # flat t*BL+b
        in_maps.append({
            "xt_d": np.ascontiguousarray(
                XT.reshape(128, NT).astype(bf16)),
            "w5a_d": np.ascontiguousarray(W5a.astype(bf16)),
            "u5_d": u5p, "w6_d": w6p, "u6_d": u6p,
            "b5_d": b5p, "b6_d": b6p,
            "br5_d": brh5, "br6_d": brh6,
            "wd_d": wdp,
            "dm_d": np.ascontiguousarray(dmc.reshape(1, NT)),
        })
    return in_maps, has_brh, float(bd.reshape(-1)[0])


def kernel(**inputs):
    from concourse.bass_utils import run_bass_kernel_spmd

    in_maps, has_brh, bd_val = _prep(inputs)
    key = (has_brh, bd_val)
    if key not in _CACHE:
        _CACHE[key] = _build(bd_val, has_brh)
    nc = _CACHE[key]
    res = run_bass_kernel_spmd(nc, in_maps, core_ids=list(range(NCORES)))
    out = np.empty((B, T, 1), np.float32)
    for cidx in range(NCORES):
        flat = res.results[cidx]["out_d"].reshape(NT)  # flat = t*BL + b
        out[cidx * BL:(cidx + 1) * BL, :, 0] = flat.reshape(T, BL).T
    return out
